# revision 3
# baseline (speedup 1.0000x reference)
"""MoE routing kernel for TRN2 (8 NeuronCores).

The reference MoE applies row 0's top-2 expert choice (indices and softmax
weights) to the entire batch, so the whole module collapses to

    out = x @ (w0*We[i0] + w1*We[i1]).T + (w0*be[i0] + w1*be[i1])

a single [16384,2048] @ [2048,2048] matmul with bias. Host does the tiny
row-0 gating and combines the two selected experts; the device runs the
matmul data-parallel over tokens (2048 tokens per core, no collectives).

Device kernel layout (per core):
  xt  [2048(d), 2048(m)]  f32r  -- x shard, pre-transposed on host
  wt  [2048(d), 2048(o)]  f32r  -- combined expert weight, transposed
  bias[128, 2048(o)]      f32   -- combined bias broadcast over partitions
  out [2048(m), 2048(o)]  f32
PSUM tile [128m, 512o] accumulates over 16 k-matmuls (float32r runs at
1 cycle/row for free dim >= 256, i.e. full PE rate for fp32 data).
"""

import os
import sys

import numpy as np

if "/opt/trn_rl_repo" not in sys.path:
    sys.path.insert(0, "/opt/trn_rl_repo")

N, D, E, TOPK = 16384, 2048, 8, 2
N_CORES = 8
M_SHARD = N // N_CORES  # 2048 tokens per core
P = 128
K_TILES = D // P        # 16 contraction slabs
M_TILES = M_SHARD // P  # 16
N_FREE = 512
N_TILES = D // N_FREE   # 4

_CACHE = {}


def _build_nc():
    import concourse.tile as tile
    from concourse import bacc, mybir

    nc = bacc.Bacc(None, target_bir_lowering=False)
    f32 = mybir.dt.float32
    f32r = mybir.dt.float32r

    xt = nc.dram_tensor("xt", [D, M_SHARD], f32r, kind="ExternalInput")
    wt = nc.dram_tensor("wt", [D, D], f32r, kind="ExternalInput")
    bias = nc.dram_tensor("bias", [P, D], f32, kind="ExternalInput")
    out = nc.dram_tensor("out", [M_SHARD, D], f32, kind="ExternalOutput")

    xt3 = xt[:, :].rearrange("(k p) m -> p k m", p=P)
    wt3 = wt[:, :].rearrange("(k p) n -> p k n", p=P)

    with tile.TileContext(nc) as tc:
        with tc.tile_pool(name="wpool", bufs=1) as wpool, \
             tc.tile_pool(name="xpool", bufs=3) as xpool, \
             tc.tile_pool(name="bpool", bufs=1) as bpool, \
             tc.tile_pool(name="opool", bufs=3) as opool, \
             tc.tile_pool(name="psum", bufs=4, space="PSUM") as psum_pool:

            bias_t = bpool.tile([P, D], f32, name="bias_t", tag="bias_t")
            nc.sync.dma_start(out=bias_t[:, :], in_=bias[:, :])

            wtiles = []
            for k in range(K_TILES):
                wk = wpool.tile([P, D], f32r, name=f"w{k}", tag=f"w{k}")
                nc.sync.dma_start(out=wk[:, :], in_=wt3[:, k, :])
                wtiles.append(wk)

            for m in range(M_TILES):
                xs = xpool.tile([P, K_TILES, P], f32r, name="xs", tag="xs")
                nc.sync.dma_start(
                    out=xs[:, :, :], in_=xt3[:, :, m * P:(m + 1) * P]
                )
                ot = opool.tile([P, D], f32, name="ot", tag="ot")
                for n in range(N_TILES):
                    ps = psum_pool.tile([P, N_FREE], f32, name="ps", tag="ps")
                    for k in range(K_TILES):
                        nc.tensor.matmul(
                            ps[:, :],
                            lhsT=xs[:, k, :],
                            rhs=wtiles[k][:, n * N_FREE:(n + 1) * N_FREE],
                            start=(k == 0),
                            stop=(k == K_TILES - 1),
                        )
                    nc.vector.tensor_add(
                        ot[:, n * N_FREE:(n + 1) * N_FREE],
                        ps[:, :],
                        bias_t[:, n * N_FREE:(n + 1) * N_FREE],
                    )
                nc.sync.dma_start(out=out[m * P:(m + 1) * P, :], in_=ot[:, :])

    nc.compile()
    return nc


def _get_nc():
    if "nc" not in _CACHE:
        _CACHE["nc"] = _build_nc()
    return _CACHE["nc"]


def _ensure_ntff_hook():
    """Register the axon NTFF profile hook (the image's antenv lacks
    axon_hooks; recreate it and wire the ctypes hook from trn_boot)."""
    import types

    try:
        from antenv.axon_hooks import get_axon_ntff_profile_hook  # noqa: F401
        return
    except ImportError:
        pass
    try:
        import antenv
        from trn_agent_boot.trn_boot import _ntff_profile_via_ctypes

        mod = types.ModuleType("antenv.axon_hooks")
        _state = {"hook": None}
        mod.set_axon_ntff_profile_hook = lambda h: _state.__setitem__("hook", h)
        mod.get_axon_ntff_profile_hook = lambda: _state["hook"]
        sys.modules["antenv.axon_hooks"] = mod
        antenv.axon_hooks = mod
        mod.set_axon_ntff_profile_hook(
            _ntff_profile_via_ctypes("/opt/axon/libaxon_pjrt.so")
        )
        # avoid the S3 artifact upload in the trace path
        import concourse.bass_utils as bu

        bu.upload_artifacts = lambda tmpdir: tmpdir
    except Exception as e:  # profiling is best-effort
        print(f"NTFF hook setup failed: {e}", file=sys.stderr)


def kernel(x, Wg, bg, We, be):
    from concourse.bass_utils import run_bass_kernel_spmd

    x = np.asarray(x, dtype=np.float32)
    Wg = np.asarray(Wg, dtype=np.float32)
    bg = np.asarray(bg, dtype=np.float32)
    We = np.asarray(We, dtype=np.float32)
    be = np.asarray(be, dtype=np.float32)

    # Row-0 gating on host (16K FLOPs): softmax over 8 logits, top-2.
    logits = x[0].astype(np.float64) @ Wg.astype(np.float64).T + bg.astype(
        np.float64
    )
    probs = np.exp(logits - logits.max())
    probs /= probs.sum()
    idx = np.argsort(-probs, kind="stable")[:TOPK]
    w0 = probs[idx]

    Wc = w0[0] * We[idx[0]].astype(np.float64) + w0[1] * We[idx[1]].astype(
        np.float64
    )
    bc = w0[0] * be[idx[0]].astype(np.float64) + w0[1] * be[idx[1]].astype(
        np.float64
    )
    wt = np.ascontiguousarray(Wc.T).astype(np.float32)
    bias = np.ascontiguousarray(
        np.broadcast_to(bc.astype(np.float32), (P, D))
    )

    nc = _get_nc()
    in_maps = []
    for c in range(N_CORES):
        xt = np.ascontiguousarray(x[c * M_SHARD:(c + 1) * M_SHARD].T)
        in_maps.append({"xt": xt, "wt": wt, "bias": bias})

    trace = bool(int(os.environ.get("KERNEL_TRACE", "0")))
    tmpdir = None
    if trace:
        import tempfile

        _ensure_ntff_hook()
        tmpdir = tempfile.mkdtemp(prefix="moe_trace_")
        _CACHE["last_tmpdir"] = tmpdir
    res = run_bass_kernel_spmd(
        nc, in_maps, core_ids=list(range(N_CORES)), trace=trace, tmpdir=tmpdir
    )
    _CACHE["last_results"] = res

    return np.concatenate(
        [res.results[c]["out"] for c in range(N_CORES)], axis=0
    )


# revision 6
# speedup vs baseline: 1.0861x; 1.0861x over previous
"""MoE routing kernel for TRN2 (8 NeuronCores).

The reference MoE applies row 0's top-2 expert choice (indices and softmax
weights) to the entire batch, so the whole module collapses to

    out = x @ (w0*We[i0] + w1*We[i1]).T + (w0*be[i0] + w1*be[i1])

a single [16384,2048] @ [2048,2048] matmul with bias. Host does the tiny
row-0 gating and combines the two selected experts; the device runs the
matmul data-parallel over tokens (2048 tokens per core, no collectives).

Per-core schedule (profile-driven):
  Stage 1: the first 4 m-tiles run k-outer, chasing the W DMA stream
           (W arrives as 16 independent [128,4,512] chunks in n-major
           order), so the PE starts ~6us in instead of waiting ~45us
           for the whole 16.8MB weight.
  Stage 2: remaining 12 m-tiles run k-inner against the resident W.
Inputs stream on the SP HWDGE queue, outputs on the Activation queue.
float32r matmuls run at 1 cycle/row (full PE rate) with ~1e-4 rel err.
"""

import os
import sys

import numpy as np

if "/opt/trn_rl_repo" not in sys.path:
    sys.path.insert(0, "/opt/trn_rl_repo")

N, D, E, TOPK = 16384, 2048, 8, 2
N_CORES = 8
M_SHARD = N // N_CORES  # 2048 tokens per core
P = 128
K_TILES = D // P        # 16 contraction slabs
M_TILES = M_SHARD // P  # 16
N_FREE = 512
N_TILES = D // N_FREE   # 4
KG = 4                  # k-slabs per W chunk / xp chunk
JG = K_TILES // KG      # 4 chunks per n-tile
M_HEAD = 4              # m-tiles computed during the W stream (stage 1)
M_SLAB = 256            # tokens per stage-2 x DMA (two m-tiles)

_CACHE = {}


def _build_nc():
    import concourse.tile as tile
    from concourse import bacc, mybir

    nc = bacc.Bacc(None, target_bir_lowering=False)
    f32 = mybir.dt.float32
    f32r = mybir.dt.float32r

    # DRAM I/O.  xp: first 512 tokens packed [j][p][kk][m512] so stage 1
    # loads 8KB-run chunks; xt: remaining tokens in [d, m] layout;
    # wt: weight packed [n][j][p][kk][512] (8KB runs per chunk).
    xp = nc.dram_tensor("xp", [JG, P, KG, M_HEAD * P], f32r, kind="ExternalInput")
    xt = nc.dram_tensor("xt", [D, M_SHARD - M_HEAD * P], f32r, kind="ExternalInput")
    wt = nc.dram_tensor("wt", [N_TILES, JG, P, KG, N_FREE], f32r, kind="ExternalInput")
    bias = nc.dram_tensor("bias", [P, D], f32, kind="ExternalInput")
    out = nc.dram_tensor("out", [M_SHARD, D], f32, kind="ExternalOutput")

    xt3 = xt[:, :].rearrange("(k p) m -> p k m", p=P)

    n_slabs = (M_SHARD - M_HEAD * P) // M_SLAB  # 6 stage-2 slabs

    with tile.TileContext(nc) as tc:
        with tc.tile_pool(name="wpool", bufs=1) as wpool, \
             tc.tile_pool(name="xppool", bufs=1) as xppool, \
             tc.tile_pool(name="xpool", bufs=2) as xpool, \
             tc.tile_pool(name="bpool", bufs=1) as bpool, \
             tc.tile_pool(name="opool", bufs=3) as opool, \
             tc.tile_pool(name="psum", bufs=2, space="PSUM") as psum_pool:

            bias_t = bpool.tile([P, D], f32, name="bias_t", tag="bias_t")
            nc.sync.dma_start(out=bias_t[:, :], in_=bias[:, :])

            # stage-1 input DMAs, interleaved so the k-chase can start
            # as soon as (xp[j], w[0][j]) land.
            xpt = []
            wc = [[None] * JG for _ in range(N_TILES)]
            for j in range(JG):
                t = xppool.tile([P, KG, M_HEAD * P], f32r, name=f"xp{j}", tag=f"xp{j}")
                nc.sync.dma_start(out=t[:, :, :], in_=xp[j])
                xpt.append(t)
                w = wpool.tile([P, KG, N_FREE], f32r, name=f"w0{j}", tag=f"w0_{j}")
                nc.sync.dma_start(out=w[:, :, :], in_=wt[0, j])
                wc[0][j] = w
            for n in range(1, N_TILES):
                for j in range(JG):
                    w = wpool.tile([P, KG, N_FREE], f32r, name=f"w{n}{j}", tag=f"w{n}_{j}")
                    nc.sync.dma_start(out=w[:, :, :], in_=wt[n, j])
                    wc[n][j] = w

            # Stage 1: m0..3, k-outer chase of the W/xp stream.
            for n in range(N_TILES):
                pss = []
                for m in range(M_HEAD):
                    ps = psum_pool.tile([P, N_FREE], f32, name=f"ps1_{n}_{m}",
                                        tag=f"ps{m}")
                    pss.append(ps)
                for j in range(JG):
                    for kk in range(KG):
                        for m in range(M_HEAD):
                            nc.tensor.matmul(
                                pss[m][:, :],
                                lhsT=xpt[j][:, kk, m * P:(m + 1) * P],
                                rhs=wc[n][j][:, kk, :],
                                start=(j == 0 and kk == 0),
                                stop=(j == JG - 1 and kk == KG - 1),
                            )
                for m in range(M_HEAD):
                    ot = opool.tile([P, N_FREE], f32, name="ot", tag="ot")
                    nc.vector.tensor_add(
                        ot[:, :], pss[m][:, :],
                        bias_t[:, n * N_FREE:(n + 1) * N_FREE],
                    )
                    nc.scalar.dma_start(
                        out=out[m * P:(m + 1) * P,
                                n * N_FREE:(n + 1) * N_FREE],
                        in_=ot[:, :],
                    )

            # Stage 2: m4..15, k-inner against resident W.
            for s in range(n_slabs):
                xs = xpool.tile([P, K_TILES, M_SLAB], f32r, name="xs", tag="xs")
                nc.sync.dma_start(
                    out=xs[:, :, :],
                    in_=xt3[:, :, s * M_SLAB:(s + 1) * M_SLAB],
                )
                for mi in range(M_SLAB // P):
                    m = M_HEAD + s * (M_SLAB // P) + mi
                    for n in range(N_TILES):
                        ps = psum_pool.tile([P, N_FREE], f32, name="ps2",
                                            tag=f"ps{(m % 2) * 2 + n % 2}")
                        for k in range(K_TILES):
                            nc.tensor.matmul(
                                ps[:, :],
                                lhsT=xs[:, k, mi * P:(mi + 1) * P],
                                rhs=wc[n][k // KG][:, k % KG, :],
                                start=(k == 0),
                                stop=(k == K_TILES - 1),
                            )
                        ot = opool.tile([P, N_FREE], f32, name="ot", tag="ot")
                        nc.vector.tensor_add(
                            ot[:, :], ps[:, :],
                            bias_t[:, n * N_FREE:(n + 1) * N_FREE],
                        )
                        nc.scalar.dma_start(
                            out=out[m * P:(m + 1) * P,
                                    n * N_FREE:(n + 1) * N_FREE],
                            in_=ot[:, :],
                        )

    nc.compile()
    return nc


def _get_nc():
    if "nc" not in _CACHE:
        _CACHE["nc"] = _build_nc()
    return _CACHE["nc"]


def _ensure_ntff_hook():
    """Register the axon NTFF profile hook (the image's antenv lacks
    axon_hooks; recreate it and wire the ctypes hook from trn_boot)."""
    import types

    try:
        from antenv.axon_hooks import get_axon_ntff_profile_hook  # noqa: F401
        return
    except ImportError:
        pass
    try:
        import antenv
        from trn_agent_boot.trn_boot import _ntff_profile_via_ctypes

        mod = types.ModuleType("antenv.axon_hooks")
        _state = {"hook": None}
        mod.set_axon_ntff_profile_hook = lambda h: _state.__setitem__("hook", h)
        mod.get_axon_ntff_profile_hook = lambda: _state["hook"]
        sys.modules["antenv.axon_hooks"] = mod
        antenv.axon_hooks = mod
        mod.set_axon_ntff_profile_hook(
            _ntff_profile_via_ctypes("/opt/axon/libaxon_pjrt.so")
        )
        # avoid the S3 artifact upload in the trace path
        import concourse.bass_utils as bu

        bu.upload_artifacts = lambda tmpdir: tmpdir
    except Exception as e:  # profiling is best-effort
        print(f"NTFF hook setup failed: {e}", file=sys.stderr)


def kernel(x, Wg, bg, We, be):
    from concourse.bass_utils import run_bass_kernel_spmd

    x = np.asarray(x, dtype=np.float32)
    Wg = np.asarray(Wg, dtype=np.float32)
    bg = np.asarray(bg, dtype=np.float32)
    We = np.asarray(We, dtype=np.float32)
    be = np.asarray(be, dtype=np.float32)

    # Row-0 gating on host (16K FLOPs): softmax over 8 logits, top-2.
    logits = x[0].astype(np.float64) @ Wg.astype(np.float64).T + bg.astype(
        np.float64
    )
    probs = np.exp(logits - logits.max())
    probs /= probs.sum()
    idx = np.argsort(-probs, kind="stable")[:TOPK]
    w0 = probs[idx]

    Wc = w0[0] * We[idx[0]].astype(np.float64) + w0[1] * We[idx[1]].astype(
        np.float64
    )
    bc = w0[0] * be[idx[0]].astype(np.float64) + w0[1] * be[idx[1]].astype(
        np.float64
    )
    WcT = np.ascontiguousarray(Wc.T).astype(np.float32)  # [d, o]
    # [n, j, p, kk, f]: d = (j kk p), o = (n f)
    wt = np.ascontiguousarray(
        WcT.reshape(JG, KG, P, N_TILES, N_FREE).transpose(3, 0, 2, 1, 4)
    )
    bias = np.ascontiguousarray(
        np.broadcast_to(bc.astype(np.float32), (P, D))
    )

    nc = _get_nc()
    in_maps = []
    mh = M_HEAD * P
    for c in range(N_CORES):
        xsh = x[c * M_SHARD:(c + 1) * M_SHARD]          # [m, d]
        xT = np.ascontiguousarray(xsh.T)                 # [d, m]
        # head tokens packed [j, p, kk, m]
        xp = np.ascontiguousarray(
            xT[:, :mh].reshape(JG, KG, P, mh).transpose(0, 2, 1, 3)
        )
        xt = np.ascontiguousarray(xT[:, mh:])
        in_maps.append({"xp": xp, "xt": xt, "wt": wt, "bias": bias})

    trace = bool(int(os.environ.get("KERNEL_TRACE", "0")))
    tmpdir = None
    if trace:
        import tempfile

        _ensure_ntff_hook()
        tmpdir = tempfile.mkdtemp(prefix="moe_trace_")
        _CACHE["last_tmpdir"] = tmpdir
    res = run_bass_kernel_spmd(
        nc, in_maps, core_ids=list(range(N_CORES)), trace=trace, tmpdir=tmpdir
    )
    _CACHE["last_results"] = res

    return np.concatenate(
        [res.results[c]["out"] for c in range(N_CORES)], axis=0
    )


# revision 7
# speedup vs baseline: 1.1273x; 1.0380x over previous
"""MoE routing kernel for TRN2 (8 NeuronCores).

The reference MoE applies row 0's top-2 expert choice (indices and softmax
weights) to the entire batch, so the whole module collapses to

    out = x @ (w0*We[i0] + w1*We[i1]).T + (w0*be[i0] + w1*be[i1])

a single [16384,2048] @ [2048,2048] matmul with bias. Host does the tiny
row-0 gating and combines the two selected experts; the device runs the
matmul data-parallel over tokens (2048 tokens per core, no collectives).

Per-core schedule (profile-driven):
  Stage 1: the first 4 m-tiles run k-outer, chasing the W DMA stream
           (W arrives as 16 independent [128,4,512] chunks in n-major
           order), so the PE starts ~6us in instead of waiting ~45us
           for the whole 16.8MB weight.
  Stage 2: remaining 12 m-tiles run k-inner against the resident W.
Inputs stream on the SP HWDGE queue, outputs on the Activation queue.
float32r matmuls run at 1 cycle/row (full PE rate) with ~1e-4 rel err.
"""

import os
import sys

import numpy as np

if "/opt/trn_rl_repo" not in sys.path:
    sys.path.insert(0, "/opt/trn_rl_repo")

N, D, E, TOPK = 16384, 2048, 8, 2
N_CORES = 8
M_SHARD = N // N_CORES  # 2048 tokens per core
P = 128
K_TILES = D // P        # 16 contraction slabs
M_TILES = M_SHARD // P  # 16
N_FREE = 512
N_TILES = D // N_FREE   # 4
KG = 4                  # k-slabs per W chunk / xp chunk
JG = K_TILES // KG      # 4 chunks per n-tile
M_HEAD = 4              # m-tiles computed during the W stream (stage 1)
M_SLAB = 256            # tokens per stage-2 x DMA (two m-tiles)

_CACHE = {}


def _build_nc():
    import concourse.tile as tile
    from concourse import bacc, mybir

    nc = bacc.Bacc(None, target_bir_lowering=False)
    f32 = mybir.dt.float32
    f32r = mybir.dt.float32r

    # DRAM I/O.  xp: first 512 tokens packed [j][p][kk][m512] so stage 1
    # loads 8KB-run chunks; xt: remaining tokens in [d, m] layout;
    # wt: weight packed [n][j][p][kk][512] (8KB runs per chunk).
    xp = nc.dram_tensor("xp", [JG, P, KG, M_HEAD * P], f32r, kind="ExternalInput")
    xt = nc.dram_tensor("xt", [D, M_SHARD - M_HEAD * P], f32r, kind="ExternalInput")
    wt = nc.dram_tensor("wt", [N_TILES, JG, P, KG, N_FREE], f32r, kind="ExternalInput")
    bias = nc.dram_tensor("bias", [P, D], f32, kind="ExternalInput")
    out = nc.dram_tensor("out", [M_SHARD, D], f32, kind="ExternalOutput")

    xt3 = xt[:, :].rearrange("(k p) m -> p k m", p=P)

    n_slabs = (M_SHARD - M_HEAD * P) // M_SLAB  # 6 stage-2 slabs

    with tile.TileContext(nc) as tc:
        with tc.tile_pool(name="wpool", bufs=1) as wpool, \
             tc.tile_pool(name="xppool", bufs=1) as xppool, \
             tc.tile_pool(name="xpool", bufs=2) as xpool, \
             tc.tile_pool(name="bpool", bufs=1) as bpool, \
             tc.tile_pool(name="opool", bufs=3) as opool, \
             tc.tile_pool(name="psum", bufs=2, space="PSUM") as psum_pool:

            # stage-1 input DMAs, interleaved so the k-chase can start
            # as soon as (w[0][j], xp[j]) land; bias only gates the first
            # eviction (~30us in) so it loads after the critical chunks.
            xpt = []
            wc = [[None] * JG for _ in range(N_TILES)]
            for j in range(JG):
                w = wpool.tile([P, KG, N_FREE], f32r, name=f"w0{j}", tag=f"w0_{j}")
                nc.sync.dma_start(out=w[:, :, :], in_=wt[0, j])
                wc[0][j] = w
                t = xppool.tile([P, KG, M_HEAD * P], f32r, name=f"xp{j}", tag=f"xp{j}")
                nc.sync.dma_start(out=t[:, :, :], in_=xp[j])
                xpt.append(t)

            bias_t = bpool.tile([P, D], f32, name="bias_t", tag="bias_t")
            nc.sync.dma_start(out=bias_t[:, :], in_=bias[:, :])

            for n in range(1, N_TILES):
                for j in range(JG):
                    w = wpool.tile([P, KG, N_FREE], f32r, name=f"w{n}{j}", tag=f"w{n}_{j}")
                    nc.sync.dma_start(out=w[:, :, :], in_=wt[n, j])
                    wc[n][j] = w

            # Stage 1: m0..3, k-outer chase of the W/xp stream.
            for n in range(N_TILES):
                pss = []
                for m in range(M_HEAD):
                    ps = psum_pool.tile([P, N_FREE], f32, name=f"ps1_{n}_{m}",
                                        tag=f"ps{m}")
                    pss.append(ps)
                for j in range(JG):
                    for kk in range(KG):
                        for m in range(M_HEAD):
                            nc.tensor.matmul(
                                pss[m][:, :],
                                lhsT=xpt[j][:, kk, m * P:(m + 1) * P],
                                rhs=wc[n][j][:, kk, :],
                                start=(j == 0 and kk == 0),
                                stop=(j == JG - 1 and kk == KG - 1),
                            )
                for m in range(M_HEAD):
                    ot = opool.tile([P, N_FREE], f32, name="ot", tag="ot")
                    nc.vector.tensor_add(
                        ot[:, :], pss[m][:, :],
                        bias_t[:, n * N_FREE:(n + 1) * N_FREE],
                    )
                    nc.scalar.dma_start(
                        out=out[m * P:(m + 1) * P,
                                n * N_FREE:(n + 1) * N_FREE],
                        in_=ot[:, :],
                    )

            # Stage 2: m4..15, k-inner against resident W.
            for s in range(n_slabs):
                xs = xpool.tile([P, K_TILES, M_SLAB], f32r, name="xs", tag="xs")
                nc.sync.dma_start(
                    out=xs[:, :, :],
                    in_=xt3[:, :, s * M_SLAB:(s + 1) * M_SLAB],
                )
                for mi in range(M_SLAB // P):
                    m = M_HEAD + s * (M_SLAB // P) + mi
                    for n in range(N_TILES):
                        ps = psum_pool.tile([P, N_FREE], f32, name="ps2",
                                            tag=f"ps{(m % 2) * 2 + n % 2}")
                        for k in range(K_TILES):
                            nc.tensor.matmul(
                                ps[:, :],
                                lhsT=xs[:, k, mi * P:(mi + 1) * P],
                                rhs=wc[n][k // KG][:, k % KG, :],
                                start=(k == 0),
                                stop=(k == K_TILES - 1),
                            )
                        ot = opool.tile([P, N_FREE], f32, name="ot", tag="ot")
                        nc.vector.tensor_add(
                            ot[:, :], ps[:, :],
                            bias_t[:, n * N_FREE:(n + 1) * N_FREE],
                        )
                        nc.scalar.dma_start(
                            out=out[m * P:(m + 1) * P,
                                    n * N_FREE:(n + 1) * N_FREE],
                            in_=ot[:, :],
                        )

    nc.compile()
    return nc


def _get_nc():
    if "nc" not in _CACHE:
        _CACHE["nc"] = _build_nc()
    return _CACHE["nc"]


def _ensure_ntff_hook():
    """Register the axon NTFF profile hook (the image's antenv lacks
    axon_hooks; recreate it and wire the ctypes hook from trn_boot)."""
    import types

    try:
        from antenv.axon_hooks import get_axon_ntff_profile_hook  # noqa: F401
        return
    except ImportError:
        pass
    try:
        import antenv
        from trn_agent_boot.trn_boot import _ntff_profile_via_ctypes

        mod = types.ModuleType("antenv.axon_hooks")
        _state = {"hook": None}
        mod.set_axon_ntff_profile_hook = lambda h: _state.__setitem__("hook", h)
        mod.get_axon_ntff_profile_hook = lambda: _state["hook"]
        sys.modules["antenv.axon_hooks"] = mod
        antenv.axon_hooks = mod
        mod.set_axon_ntff_profile_hook(
            _ntff_profile_via_ctypes("/opt/axon/libaxon_pjrt.so")
        )
        # avoid the S3 artifact upload in the trace path
        import concourse.bass_utils as bu

        bu.upload_artifacts = lambda tmpdir: tmpdir
    except Exception as e:  # profiling is best-effort
        print(f"NTFF hook setup failed: {e}", file=sys.stderr)


def kernel(x, Wg, bg, We, be):
    from concourse.bass_utils import run_bass_kernel_spmd

    x = np.asarray(x, dtype=np.float32)
    Wg = np.asarray(Wg, dtype=np.float32)
    bg = np.asarray(bg, dtype=np.float32)
    We = np.asarray(We, dtype=np.float32)
    be = np.asarray(be, dtype=np.float32)

    # Row-0 gating on host (16K FLOPs): softmax over 8 logits, top-2.
    logits = x[0].astype(np.float64) @ Wg.astype(np.float64).T + bg.astype(
        np.float64
    )
    probs = np.exp(logits - logits.max())
    probs /= probs.sum()
    idx = np.argsort(-probs, kind="stable")[:TOPK]
    w0 = probs[idx]

    Wc = w0[0] * We[idx[0]].astype(np.float64) + w0[1] * We[idx[1]].astype(
        np.float64
    )
    bc = w0[0] * be[idx[0]].astype(np.float64) + w0[1] * be[idx[1]].astype(
        np.float64
    )
    WcT = np.ascontiguousarray(Wc.T).astype(np.float32)  # [d, o]
    # [n, j, p, kk, f]: d = (j kk p), o = (n f)
    wt = np.ascontiguousarray(
        WcT.reshape(JG, KG, P, N_TILES, N_FREE).transpose(3, 0, 2, 1, 4)
    )
    bias = np.ascontiguousarray(
        np.broadcast_to(bc.astype(np.float32), (P, D))
    )

    nc = _get_nc()
    in_maps = []
    mh = M_HEAD * P
    for c in range(N_CORES):
        xsh = x[c * M_SHARD:(c + 1) * M_SHARD]          # [m, d]
        xT = np.ascontiguousarray(xsh.T)                 # [d, m]
        # head tokens packed [j, p, kk, m]
        xp = np.ascontiguousarray(
            xT[:, :mh].reshape(JG, KG, P, mh).transpose(0, 2, 1, 3)
        )
        xt = np.ascontiguousarray(xT[:, mh:])
        in_maps.append({"xp": xp, "xt": xt, "wt": wt, "bias": bias})

    trace = bool(int(os.environ.get("KERNEL_TRACE", "0")))
    tmpdir = None
    if trace:
        import tempfile

        _ensure_ntff_hook()
        tmpdir = tempfile.mkdtemp(prefix="moe_trace_")
        _CACHE["last_tmpdir"] = tmpdir
    res = run_bass_kernel_spmd(
        nc, in_maps, core_ids=list(range(N_CORES)), trace=trace, tmpdir=tmpdir
    )
    _CACHE["last_results"] = res

    return np.concatenate(
        [res.results[c]["out"] for c in range(N_CORES)], axis=0
    )
